# revision 1
# baseline (speedup 1.0000x reference)
"""Trainium2 Bass kernel for nn_CGNN (gnn_message_passing).

Strategy
--------
The per-edge gather/scatter-add over a shared edge list is algebraically a
dense matmul: messages[b] = A @ h_new[b] with A[n, m] = sum_{e: dst=n, src=m}
w_e (A is [128, 128], shared across batch and layers).  The whole network is
then dense matmuls + relu, executed per-sample as [128, 256] tiles:

  - h is stored per-sample TRANSPOSED (hT: feature on partitions, node on
    free dim) in one resident SBUF buffer.  The layer matmul z = h @ W uses
    hT chunks as the stationary operand (lhsT), which re-transposes for free:
    z comes out in normal [node, feature] layout.
  - messages^T = h_new^T-producing matmul: lhsT = h_new (normal layout, from
    the relu eviction of z), rhs = A^T.  Output mT is in hT layout.
  - The residual h + messages is accumulated on the PE itself: an identity
    matmul adds hT into the mT PSUM accumulation; a single DVE
    relu-max eviction then produces the next layer's hT (bf16).
  - The encoder h0 = x*enc_w + enc_b is never materialized: layer 1 uses
    z1 = x (x) u + 1 (x) c  (u = enc_w @ W1, c = enc_b @ W1 + b1, computed on
    host in fp64), and h0's residual contribution enters the layer-1 mT PSUM
    as a rank-2 matmul from a [x; 1] stationary tile.
  - The classifier hidden = relu(h3.flat @ cls_w1 + b1) reads the resident
    hT buffer with strided APs (no transposes): for each 128-row chunk of
    cls_w1, rhs = hT[h-partitions, batch-strided free].  cls_w1 is streamed
    from HBM in bf16.

Data-parallel across 8 cores over the batch axis (256 samples/core).
All matmuls in fp16 (fp32 PSUM accumulation).
"""

import sys

for _p in ("/opt/trn_rl_repo",):
    if _p not in sys.path:
        sys.path.insert(0, _p)

from contextlib import ExitStack

import ml_dtypes
import numpy as np

import concourse.bacc as bacc
import concourse.bass as bass
import concourse.tile as tile
from concourse import mybir
from concourse.bass_utils import run_bass_kernel_spmd

dt = mybir.dt
AF = mybir.ActivationFunctionType
F16 = np.float16

B, N, H, NL, OUT = 2048, 128, 256, 3, 2
N_CORES = 8
BC = B // N_CORES            # samples per core (256)
G = 4                        # samples per elementwise eviction group
NG = BC // G
KB = 8                       # cls_w1 128-row chunks per DMA (512 KB each)
LXB = (BC + 2) // 3          # lhsx col blocks (3 samples per block, bases 0/32/64)
N_CHUNKS = (N * H) // 128    # 256 contraction chunks in the classifier

_BUILT = {}


def _build_nc(has_lbias: bool, bc: int = BC, mode: str = "full",
              repeat: int = 1):
    """Emit the Tile kernel. has_lbias: include the (rare) nonzero
    layer-bias rank-1 accumulations for layers 2..3.
    mode: "full" | "layers" (skip classifier, dump ht) | "cls"
    (skip layers, classifier reads zero-init ht)."""
    ng = bc // G
    lxb = (bc + 2) // 3
    n_chunks = N_CHUNKS
    nc = bacc.Bacc("TRN2", target_bir_lowering=False)

    lhsx_d = nc.dram_tensor("lhsx", [128, lxb * 128], dt.float16,
                            kind="ExternalInput")
    u2c_d = nc.dram_tensor("u2c", [128, 3 * H], dt.float16,
                           kind="ExternalInput")
    ew_d = nc.dram_tensor("ew", [128, 3 * H], dt.float16,
                          kind="ExternalInput")
    w23_d = nc.dram_tensor("w23", [NL - 1, H, H], dt.float16,
                           kind="ExternalInput")
    at_d = nc.dram_tensor("at_", [N, N], dt.float16, kind="ExternalInput")
    eye_d = nc.dram_tensor("eye", [128, 128], dt.float16,
                           kind="ExternalInput")
    w1_d = nc.dram_tensor("w1", [N * H, H], dt.float16, kind="ExternalInput")
    w2_d = nc.dram_tensor("w2r", [128, 2 * OUT], dt.float16,
                          kind="ExternalInput")
    cb1_d = nc.dram_tensor("cb1", [128, 2], dt.float32, kind="ExternalInput")
    cb2_d = nc.dram_tensor("cb2b", [128, OUT], dt.float32,
                           kind="ExternalInput")
    if has_lbias:
        xb_d = nc.dram_tensor("xb23", [128, 3 * (NL - 1) * H], dt.float16,
                              kind="ExternalInput")
    if mode in ("layers", "l0", "l1"):
        out_d = nc.dram_tensor("htdump", [128, bc * H], dt.float16,
                               kind="ExternalOutput")
    else:
        out_d = nc.dram_tensor("logits", [bc, OUT], dt.float32,
                               kind="ExternalOutput")

    with tile.TileContext(nc) as tc, ExitStack() as ctx:
        const = ctx.enter_context(tc.tile_pool(name="const", bufs=1))
        htp = ctx.enter_context(tc.tile_pool(name="ht", bufs=1))

        lhsx = const.tile([128, lxb * 128], dt.float16)
        u2c = const.tile([128, 3 * H], dt.float16)
        ew = const.tile([128, 3 * H], dt.float16)
        w23 = const.tile([128, (NL - 1) * 2 * H], dt.float16)
        at_t = const.tile([N, N], dt.float16)
        eye = const.tile([128, 128], dt.float16)
        w2 = const.tile([128, 2 * OUT], dt.float16)
        cb1 = const.tile([128, 2], dt.float32)
        cb2 = const.tile([128, OUT], dt.float32)

        nc.sync.dma_start(lhsx[:], lhsx_d[:])
        nc.sync.dma_start(u2c[:], u2c_d[:])
        nc.sync.dma_start(ew[:], ew_d[:])
        for li in range(NL - 1):
            for hc in range(2):
                nc.sync.dma_start(
                    w23[:, (li * 2 + hc) * H:(li * 2 + hc + 1) * H],
                    w23_d[li, hc * 128:(hc + 1) * 128, :])
        nc.sync.dma_start(at_t[:], at_d[:])
        nc.sync.dma_start(eye[:], eye_d[:])
        nc.sync.dma_start(w2[:], w2_d[:])
        nc.sync.dma_start(cb1[:], cb1_d[:])
        nc.sync.dma_start(cb2[:], cb2_d[:])
        if has_lbias:
            xb = const.tile([128, 3 * (NL - 1) * H], dt.float16)
            nc.sync.dma_start(xb[:], xb_d[:])

        # resident h (hT layout): sample s chunk hc at cols s*256 + hc*128
        ht = htp.tile([128, bc * H], dt.float16)

        def lx_ap(s):
            cb = s // 3
            return lhsx[:, cb * 128:(cb + 1) * 128]

        if mode == "cls":
            nc.vector.memset(ht[:], 0.5)
        # ---------------- phase 1: 3 GNN layers ----------------
        for _rep in range(repeat):
         if mode != "cls":
           with (
               tc.tile_pool(name="hn", bufs=3) as hnp,
               tc.tile_pool(name="zp", bufs=2, space="PSUM") as zp,
               tc.tile_pool(name="mp", bufs=2, space="PSUM") as mp,
           ):
               def emit_z(l, g):
                   z = zp.tile([128, G * H], dt.float32)
                   for si in range(G):
                       s = g * G + si
                       zs = z[:, si * H:(si + 1) * H]
                       if l == 0:
                           bi = s % 3
                           nc.tensor.matmul(
                               zs, lx_ap(s), u2c[:, bi * H:(bi + 1) * H],
                               start=True, stop=True)
                       else:
                           for hc in range(2):
                               last = (hc == 1) and not has_lbias
                               nc.tensor.matmul(
                                   zs,
                                   ht[:, s * H + hc * 128:s * H + (hc + 1) * 128],
                                   w23[:, ((l - 1) * 2 + hc) * H:
                                       ((l - 1) * 2 + hc + 1) * H],
                                   start=(hc == 0), stop=last)
                           if has_lbias:
                               bi = s % 3
                               blk = bi * (NL - 1) + (l - 1)
                               nc.tensor.matmul(
                                   zs, lx_ap(s),
                                   xb[:, blk * H:(blk + 1) * H],
                                   start=False, stop=True)
                   hn = hnp.tile([128, G * H], dt.float16)
                   nc.scalar.activation(hn[:], z[:], AF.Relu)
                   return hn

               def emit_m(l, g, hn):
                   m = mp.tile([128, G * H], dt.float32)
                   if l == 0:
                       for si in range(G):
                           s = g * G + si
                           for kc in range(2):
                               ms = m[:, si * H + kc * 128:
                                      si * H + (kc + 1) * 128]
                               nc.tensor.matmul(
                                   ms,
                                   hn[:, si * H + kc * 128:
                                      si * H + (kc + 1) * 128],
                                   at_t[:], start=True, stop=False)
                               bi = s % 3
                               blk = bi * 2 + kc
                               nc.tensor.matmul(
                                   ms, ew[:, blk * 128:(blk + 1) * 128],
                                   lx_ap(s), start=False, stop=True)
                   else:
                       # Residual first: one identity matmul per sample PAIR
                       # covers a full psum bank (start=True zeroes the whole
                       # 2KB zero-region, so it must open the group); the A
                       # matmuls then accumulate on top.
                       for p in range(G // 2):
                           nc.tensor.matmul(
                               m[:, p * 512:(p + 1) * 512], eye[:],
                               ht[:, (g * G + 2 * p) * H:
                                  (g * G + 2 * p + 2) * H],
                               start=True, stop=False)
                           for sj in range(2):
                               si = 2 * p + sj
                               for kc in range(2):
                                   nc.tensor.matmul(
                                       m[:, si * H + kc * 128:
                                         si * H + (kc + 1) * 128],
                                       hn[:, si * H + kc * 128:
                                          si * H + (kc + 1) * 128],
                                       at_t[:], start=False,
                                       stop=(sj == 1 and kc == 1))
                   # relu(h + m) -> next h (fp16), one DVE pass per group
                   nc.vector.tensor_scalar_max(
                       ht[:, g * G * H:(g + 1) * G * H], m[:], 0.0)

               if mode == "l0":
                   layer_list = [0]
               elif mode == "l1":
                   nc.vector.memset(ht[:], 0.25)
                   layer_list = [1]
               else:
                   layer_list = list(range(NL))
               for l in layer_list:
                   pend = None
                   for g in range(ng):
                       hn = emit_z(l, g)
                       if pend is not None:
                           emit_m(l, pend[0], pend[1])
                       pend = (g, hn)
                   emit_m(l, pend[0], pend[1])

         if mode in ("layers", "l0", "l1"):
             nc.sync.dma_start(out_d[:], ht[:])
         # ---------------- phase 2: classifier ----------------
         ht_v = ht[:].rearrange("p (s c) -> p s c", c=H)  # [128, BC, 256]
         if mode in ("full", "cls"):
           with (
             tc.tile_pool(name="w1p", bufs=3) as w1p,
             tc.tile_pool(name="hs", bufs=1) as hsp,
             tc.tile_pool(name="cp", bufs=1, space="PSUM") as cp,
             tc.tile_pool(name="lp", bufs=2, space="PSUM") as lp,
           ):
               hid0 = cp.tile([128, bc], dt.float32, tag="hid0")
               hid1 = cp.tile([128, bc], dt.float32, tag="hid1")
               hids = (hid0, hid1)
               w1_v = w1_d[:].rearrange("(a p) k -> p a k", p=128)
               for mc in range(n_chunks // KB):
                   w1t = w1p.tile([128, KB * H], dt.float16)
                   nc.sync.dma_start(
                       w1t[:].rearrange("p (a k) -> p a k", a=KB),
                       w1_v[:, mc * KB:(mc + 1) * KB, :])
                   for j in range(KB):
                       chunk = mc * KB + j
                       n_idx, hc = chunk // 2, chunk % 2
                       rhs = ht_v[:, :, hc * 128 + n_idx]
                       for kt in range(2):
                           nc.tensor.matmul(
                               hids[kt][:],
                               w1t[:, j * H + kt * 128:j * H + (kt + 1) * 128],
                               rhs, start=(chunk == 0),
                               stop=(chunk == n_chunks - 1))

               hidsb = hsp.tile([128, 2 * bc], dt.float16)
               for kt in range(2):
                   nc.scalar.activation(
                       hidsb[:, kt * bc:(kt + 1) * bc], hids[kt][:],
                       AF.Relu, bias=cb1[:, kt:kt + 1])

               out_v = out_d[:].rearrange("(t p) j -> t p j", p=min(128, bc))
               for bt in range(bc // min(128, bc)):
                   lg = lp.tile([128, OUT], dt.float32)
                   bw = min(128, bc)
                   for kc in range(2):
                       nc.tensor.matmul(
                           lg[:bw, :],
                           hidsb[:, kc * bc + bt * bw:kc * bc + (bt + 1) * bw],
                           w2[:, kc * OUT:(kc + 1) * OUT],
                           start=(kc == 0), stop=(kc == 1))
                   lgs = hsp.tile([128, OUT], dt.float32, tag=f"lgs{bt}")
                   nc.vector.tensor_tensor(
                       lgs[:bw, :], lg[:bw, :], cb2[:bw, :],
                       op=mybir.AluOpType.add)
                   nc.sync.dma_start(out_v[bt], lgs[:bw, :])

    nc.compile()
    return nc


def _get_nc(has_lbias: bool, bc: int = BC):
    key = (has_lbias, bc)
    if key not in _BUILT:
        _BUILT[key] = _build_nc(has_lbias, bc)
    return _BUILT[key]


def _host_arrays(x, edge_attr, enc_w, enc_b, layer_w, layer_b,
                 cls_w1, cls_b1, cls_w2, cls_b2, edge_index):
    f64 = np.float64
    src, dst = edge_index[0], edge_index[1]
    A = np.zeros((N, N), f64)
    np.add.at(A, (dst, src), edge_attr[:, 0].astype(f64))
    at_np = A.T.astype(F16)                       # rhs [m, n] = A[n, m]

    u = enc_w[0].astype(f64) @ layer_w[0].astype(f64)
    c = enc_b.astype(f64) @ layer_w[0].astype(f64) + layer_b[0].astype(f64)

    # u2c3: block bi has [u; c] only at rows 32bi, 32bi+1 (zero elsewhere)
    u2c_np = np.zeros((128, 3 * H), np.float32)
    for bi in range(3):
        u2c_np[32 * bi, bi * H:(bi + 1) * H] = u
        u2c_np[32 * bi + 1, bi * H:(bi + 1) * H] = c
    u2c_np = u2c_np.astype(F16)
    # ew3: block (bi, kc) has [enc_w chunk; enc_b chunk] at rows 32bi, +1
    ew_np = np.zeros((128, 3 * 2 * 128), np.float32)
    for bi in range(3):
        for kc in range(2):
            blk = bi * 2 + kc
            ew_np[32 * bi, blk * 128:(blk + 1) * 128] = \
                enc_w[0][kc * 128:(kc + 1) * 128]
            ew_np[32 * bi + 1, blk * 128:(blk + 1) * 128] = \
                enc_b[kc * 128:(kc + 1) * 128]
    ew_np = ew_np.astype(F16)

    w23_np = layer_w[1:].astype(F16)
    eye_np = np.eye(128, dtype=np.float32).astype(F16)
    w1_np = cls_w1.astype(F16)
    w2_np = np.ascontiguousarray(
        cls_w2.reshape(2, 128, OUT).transpose(1, 0, 2).reshape(128, 2 * OUT)
    ).astype(F16)
    cb1_np = np.ascontiguousarray(cls_b1.reshape(2, 128).T).astype(np.float32)
    cb2_np = np.tile(cls_b2.astype(np.float32), (128, 1))

    has_lbias = bool(np.any(layer_b[1:] != 0))
    xb_np = None
    if has_lbias:
        xbt = np.zeros((128, 3 * (NL - 1) * H), np.float32)
        for bi in range(3):
            for li in range(NL - 1):
                blk = bi * (NL - 1) + li
                xbt[32 * bi + 1, blk * H:(blk + 1) * H] = layer_b[li + 1]
        xb_np = xbt.astype(F16)

    def lhsx_for(x_core):                          # x_core [BC, 128] fp32
        t = np.zeros((128, LXB * 128), np.float32)
        for s in range(BC):
            bi, cb = s % 3, s // 3
            t[32 * bi, cb * 128:(cb + 1) * 128] = x_core[s]
            t[32 * bi + 1, cb * 128:(cb + 1) * 128] = 1.0
        return t.astype(F16)

    shared = {
        "u2c": u2c_np, "ew": ew_np, "w23": w23_np, "at_": at_np,
        "eye": eye_np, "w1": w1_np, "w2r": w2_np, "cb1": cb1_np,
        "cb2b": cb2_np,
    }
    if has_lbias:
        shared["xb23"] = xb_np
    return shared, lhsx_for, has_lbias


def kernel(x, edge_attr, enc_w, enc_b, layer_w, layer_b,
           cls_w1, cls_b1, cls_w2, cls_b2, edge_index):
    args = [np.asarray(a) for a in (
        x, edge_attr, enc_w, enc_b, layer_w, layer_b,
        cls_w1, cls_b1, cls_w2, cls_b2, edge_index)]
    (x, edge_attr, enc_w, enc_b, layer_w, layer_b,
     cls_w1, cls_b1, cls_w2, cls_b2, edge_index) = args

    shared, lhsx_for, has_lbias = _host_arrays(
        x, edge_attr, enc_w, enc_b, layer_w, layer_b,
        cls_w1, cls_b1, cls_w2, cls_b2, edge_index)
    nc = _get_nc(has_lbias)

    in_maps = []
    for cid in range(N_CORES):
        xc = x[cid * BC:(cid + 1) * BC].astype(np.float32)
        m = dict(shared)
        m["lhsx"] = lhsx_for(xc)
        in_maps.append(m)

    res = run_bass_kernel_spmd(nc, in_maps, core_ids=list(range(N_CORES)))
    out = np.concatenate([res.results[c]["logits"] for c in range(N_CORES)],
                         axis=0)
    return out.astype(np.float32)


if __name__ == "__main__":
    rng = np.random.default_rng(0)
    ins = {
        "x": rng.standard_normal((B, N), dtype=np.float32),
        "edge_attr": rng.random((4096, 1), dtype=np.float32),
        "enc_w": rng.standard_normal((1, H), dtype=np.float32) * 0.02,
        "enc_b": np.zeros((H,), np.float32),
        "layer_w": rng.standard_normal((NL, H, H), dtype=np.float32) * 0.02,
        "layer_b": np.zeros((NL, H), np.float32),
        "cls_w1": rng.standard_normal((H * N, H), dtype=np.float32) * 0.02,
        "cls_b1": np.zeros((H,), np.float32),
        "cls_w2": rng.standard_normal((H, OUT), dtype=np.float32) * 0.02,
        "cls_b2": np.zeros((OUT,), np.float32),
        "edge_index": rng.integers(0, N, (2, 4096)).astype(np.int32),
    }
    out = kernel(**ins)
    print("kernel ran, out:", out.shape, out.dtype, np.abs(out).max())

